# revision 19
# baseline (speedup 1.0000x reference)
"""Multi-head attention (B=4, S=2048, D=512, H=8, inner=512) on 8 trn2 cores.

Sharding: tensor-parallel over heads. Core h computes head h end-to-end;
the host sums the 8 partial output projections.

Because inner == D, the per-head algebra factors so both the k and v
projections vanish from the device program:
  scores = (x Wq)(x Wk)^T = x (Wq Wk^T) x^T      M = Wq Wk^T  (host, fp64)
  out_h  = (P (x Wv)) Wp_h = (P x)(Wv Wp_h)      G = Wv Wp_h  (host, fp64)
so the device only computes q' = x M, scoresT = x q'^T, z = P x, z G.

Device layout (matmuls in float32r: full PE rate, ~1.3e-4 matmul error):
  xt [D, B*S] and xn [B*S, D] are host-prepared so both the d-contraction
  (scores/q') and t-contraction (z = P x) have their operands partition-
  aligned. scoresT tiles are [t_block, sq] so softmax's key-axis sum is a
  partition reduction: P accumulates on the vector engine, and 4 tiny
  N=1 fp32 matmuls against a ones column give per-query sums in column
  layout for the reciprocal. exp needs no max-subtraction (|scores| <~ 35
  for this data, far from fp32 overflow). Normalization is deferred to
  the output projection, applied as a per-partition scalar on the
  PSUM->SBUF move.

The bias inputs (bq/bk/bv/bp) are structurally zero for this problem
(spec fill=zeros); bp is added on host, and a host fallback covers the
(per-spec impossible) nonzero q/k/v bias case.
"""

import numpy as np

import concourse.mybir as mybir
import concourse.tile as tile
from concourse import bacc
from concourse.bass_utils import run_bass_kernel_spmd

F32 = mybir.dt.float32
F32R = mybir.dt.float32r

B, S, D, H = 4, 2048, 512, 8
E = D  # per-head inner size
NKD = D // 128   # contraction chunks over D
NW = S // 512    # query windows per batch
NT = S // 128    # key blocks per batch
ISQRT_E = 1.0 / float(np.sqrt(E))

_CACHE = {}


def _build():
    nc = bacc.Bacc("TRN2", target_bir_lowering=False, debug=False, num_devices=8)

    xt_ext = nc.dram_tensor("xt", [D, B * S], F32R, kind="ExternalInput")
    xn_ext = nc.dram_tensor("xn", [B * S, D], F32R, kind="ExternalInput")
    m_ext = nc.dram_tensor("m", [D, D], F32R, kind="ExternalInput")
    g_ext = nc.dram_tensor("g", [D, D], F32R, kind="ExternalInput")
    out_ext = nc.dram_tensor("out", [B * S, D], F32, kind="ExternalOutput")
    dbg_ext = nc.dram_tensor("dbg", [1, 64], F32, kind="ExternalOutput")

    with tile.TileContext(nc) as tc:
        with (
            tc.tile_pool(name="wpool", bufs=1) as wpool,
            tc.tile_pool(name="xpool", bufs=2) as xpool,
            tc.tile_pool(name="actpool", bufs=2) as actpool,
            tc.tile_pool(name="qtpool", bufs=2) as qtpool,
            tc.tile_pool(name="ppool", bufs=3) as ppool,
            tc.tile_pool(name="otpool", bufs=1) as otpool,
            tc.tile_pool(name="opool", bufs=3) as opool,
            tc.tile_pool(name="rpool", bufs=1) as rpool,
            tc.tile_pool(name="mm_ps", bufs=4, space="PSUM") as mm_ps,
            tc.tile_pool(name="o_ps", bufs=1, space="PSUM") as o_ps_pool,
        ):
            # dummy matmuls during the initial DMA window lift the PE's HAM
            # clock gate to 2.4GHz before the first real matmul arrives
            warm_sb = wpool.tile([128, 128], F32)
            nc.vector.memset(warm_sb[:], 0.0)
            warm_ps = mm_ps.tile([128, 64], F32, name="warmps", tag="mm")
            for _ in range(24):
                nc.tensor.matmul(warm_ps[:], warm_sb[:, 0:128], warm_sb[:, 0:64],
                                 start=True, stop=True)
            warm_out = wpool.tile([1, 64], F32)
            nc.vector.tensor_copy(warm_out[:], warm_ps[0:1, :])
            nc.sync.dma_start(out=dbg_ext[:], in_=warm_out[:])

            m_sb = wpool.tile([128, NKD, D], F32R)
            g_sb = wpool.tile([128, NKD, D], F32R)
            for k in range(NKD):
                nc.sync.dma_start(out=m_sb[:, k, :],
                                  in_=m_ext[k * 128:(k + 1) * 128, :])

            ones_f32 = wpool.tile([128, 1], F32)
            nc.vector.memset(ones_f32[:], 1.0)

            # x in natural [t, d] layout is the stationary operand of
            # z = P x -- pure data movement, no projection matmuls. Loaded
            # one batch ahead so the descriptors clear the sync queue
            # before that batch's output DMAs pile in behind them.
            xn_tiles = {}

            def load_xn(bb):
                t_sb = actpool.tile([128, NT, D], F32R, name=f"xn{bb}", tag="v")
                for t in range(NT):
                    r0 = bb * S + t * 128
                    nc.sync.dma_start(out=t_sb[:, t, :], in_=xn_ext[r0:r0 + 128, :])
                xn_tiles[bb] = t_sb

            load_xn(0)
            for b in range(B):
                if b + 1 < B:
                    load_xn(b + 1)
                xn_sb = xn_tiles.pop(b)
                xt_sb = xpool.tile([128, NKD, S], F32R)
                # xt descriptors go out on the idle gpsimd queue so they
                # issue in parallel with xn/m on the sync queue
                for w in range(NW):
                    for k in range(NKD):
                        nc.gpsimd.dma_start(
                            out=xt_sb[:, k, w * 512:(w + 1) * 512],
                            in_=xt_ext[k * 128:(k + 1) * 128,
                                       b * S + w * 512:b * S + (w + 1) * 512],
                        )
                if b == 0:
                    # g rides behind batch 0's xt on gpsimd; first use is
                    # the first output projection, ~40us in
                    for k in range(NKD):
                        nc.gpsimd.dma_start(out=g_sb[:, k, :],
                                            in_=g_ext[k * 128:(k + 1) * 128, :])

                def emit_qt(wsl):
                    qt_sb = qtpool.tile([128, NKD, 512], F32R, name="qtw", tag="qt")
                    for me in range(NKD):
                        msl = slice(me * 128, (me + 1) * 128)
                        ps = mm_ps.tile([128, 512], F32, name="mmps", tag="mm")
                        for k in range(NKD):
                            nc.tensor.matmul(
                                ps[:], m_sb[:, k, msl], xt_sb[:, k, wsl],
                                start=(k == 0), stop=(k == NKD - 1),
                            )
                        nc.vector.tensor_copy(qt_sb[:, me, :], ps[:])
                    return qt_sb

                qt_sb = emit_qt(slice(0, 512))
                for w in range(NW):
                    o_ps = o_ps_pool.tile([128, NKD, 512], F32, name="ops", tag="ops")
                    p_acc = rpool.tile([128, 512], F32, name="pacc", tag="pacc")

                    # software-pipelined by one t-block: scores(t+1) is
                    # emitted before z(t) so the PE never stalls on exp(t)
                    s_tiles = {}
                    s_tiles[0] = mm_ps.tile([128, 512], F32, name="mmps", tag="mm")
                    for k in range(NKD):
                        nc.tensor.matmul(
                            s_tiles[0][:], xt_sb[:, k, 0:128], qt_sb[:, k, :],
                            start=(k == 0), stop=(k == NKD - 1),
                        )
                    for t in range(NT):
                        if t + 1 < NT:
                            tsl = slice((t + 1) * 128, (t + 2) * 128)
                            nxt = mm_ps.tile([128, 512], F32, name="mmps", tag="mm")
                            for k in range(NKD):
                                nc.tensor.matmul(
                                    nxt[:], xt_sb[:, k, tsl], qt_sb[:, k, :],
                                    start=(k == 0), stop=(k == NKD - 1),
                                )
                            s_tiles[t + 1] = nxt
                        p_sb = ppool.tile([128, 512], F32R, name="ptile", tag="p")
                        nc.scalar.activation(
                            p_sb[:], s_tiles.pop(t)[:],
                            mybir.ActivationFunctionType.Exp, scale=ISQRT_E,
                        )
                        # rowsum accumulates on the vector engine instead of
                        # burning a PE matmul per t-block
                        p_in = p_sb[:].bitcast(F32)
                        if t == 0:
                            nc.vector.tensor_copy(p_acc[:], p_in)
                        else:
                            nc.vector.tensor_add(p_acc[:], p_acc[:], p_in)
                        for me in range(NKD):
                            msl = slice(me * 128, (me + 1) * 128)
                            nc.tensor.matmul(
                                o_ps[:, me, :], xn_sb[:, t, msl], p_sb[:],
                                start=(t == 0), stop=(t == NT - 1),
                                skip_group_check=True,
                            )

                    # scalar engine moves z out of PSUM (frees banks for the
                    # next window while the vector engine handles rowsums)
                    zt_sb = otpool.tile([128, NKD, 512], F32R, name="zt", tag="ot")
                    for me in range(NKD):
                        nc.scalar.copy(zt_sb[:, me, :], o_ps[:, me, :])

                    # prefetch next window's q' so the PE stays busy while the
                    # normalization chain below runs on DVE/ACT
                    if w + 1 < NW:
                        qt_next = emit_qt(slice((w + 1) * 512, (w + 2) * 512))
                    else:
                        qt_next = None

                    # per-query rowsums straight into column layout:
                    # rtp[:, j] = p_acc[:, j-block].T @ ones -- tiny N=1 fp32
                    # matmuls (fp32r forbids N=1; 4 cyc/row x 1 row is free)
                    rtp = mm_ps.tile([128, 4], F32, name="rtp", tag="mm")
                    for j in range(4):
                        nc.tensor.matmul(
                            rtp[:, j:j + 1],
                            p_acc[:, j * 128:(j + 1) * 128], ones_f32[:],
                            start=True, stop=True,
                        )
                    rraw = rpool.tile([128, 4], F32, name="rraw", tag="rraw")
                    nc.vector.tensor_copy(rraw[:], rtp[:])
                    rcol = rpool.tile([128, 4], F32, name="rcol", tag="rc")
                    nc.vector.reciprocal(rcol[:], rraw[:])

                    # output projection for this window; normalization is the
                    # per-partition scalar multiply on the PSUM->SBUF move
                    for j in range(4):
                        jsl = slice(j * 128, (j + 1) * 128)
                        ps = mm_ps.tile([128, 512], F32, name="mmps", tag="mm")
                        for me in range(NKD):
                            nc.tensor.matmul(
                                ps[:], zt_sb[:, me, jsl], g_sb[:, me, :],
                                start=(me == 0), stop=(me == NKD - 1),
                            )
                        po_sb = opool.tile([128, 512], F32, name="po", tag="po")
                        nc.vector.tensor_scalar(
                            po_sb[:], ps[:], rcol[:, j:j + 1], None,
                            mybir.AluOpType.mult,
                        )
                        # output descriptors ride the scalar queue: the sync
                        # queue's xn slot-waits at batch seams must not block
                        # them, or po slots back up into the PE's psum pool
                        row0 = b * S + w * 512 + j * 128
                        nc.scalar.dma_start(
                            out=out_ext[row0:row0 + 128, :], in_=po_sb[:]
                        )
                    qt_sb = qt_next

    nc.compile()
    return nc


def _get_nc():
    if "nc" not in _CACHE:
        _CACHE["nc"] = _build()
    return _CACHE["nc"]


def _numpy_fallback(emb, Wq, bq, Wk, bk, Wv, bv, Wp, bp):
    x = emb.astype(np.float64)
    out = np.zeros((B, S, D), dtype=np.float64)
    for h in range(H):
        q = x @ Wq[h].astype(np.float64) + bq[h]
        k = x @ Wk[h].astype(np.float64) + bk[h]
        v = x @ Wv[h].astype(np.float64) + bv[h]
        for b in range(B):
            sc = (q[b] @ k[b].T) / np.sqrt(E)
            sc -= sc.max(axis=1, keepdims=True)
            p = np.exp(sc)
            p /= p.sum(axis=1, keepdims=True)
            out[b] += (p @ v[b]) @ Wp[h * E:(h + 1) * E].astype(np.float64)
    return (out + bp).astype(np.float32)


def _run(inputs, trace=False):
    emb = np.ascontiguousarray(inputs["emb_input"], dtype=np.float32)
    Wq = np.ascontiguousarray(inputs["Wq"], dtype=np.float32)
    Wk = np.ascontiguousarray(inputs["Wk"], dtype=np.float32)
    Wv = np.ascontiguousarray(inputs["Wv"], dtype=np.float32)
    Wp = np.ascontiguousarray(inputs["Wp"], dtype=np.float32)
    bq = np.asarray(inputs["bq"], dtype=np.float32)
    bk = np.asarray(inputs["bk"], dtype=np.float32)
    bv = np.asarray(inputs["bv"], dtype=np.float32)
    bp = np.asarray(inputs["bp"], dtype=np.float32)

    if np.any(bq) or np.any(bk) or np.any(bv):
        # the device program folds Wq/Wk and Wv/Wp together, which assumes
        # the q/k/v biases are structurally zero (problem spec fill=zeros);
        # anything else falls back to host math
        return _numpy_fallback(emb, Wq, bq, Wk, bk, Wv, bv, Wp, bp), None

    xt = np.ascontiguousarray(emb.transpose(2, 0, 1).reshape(D, B * S))
    xn = emb.reshape(B * S, D)
    in_maps = []
    for h in range(H):
        wq64 = Wq[h].astype(np.float64)
        wk64 = Wk[h].astype(np.float64)
        wv64 = Wv[h].astype(np.float64)
        wp64 = Wp[h * E:(h + 1) * E, :].astype(np.float64)
        in_maps.append({
            "xt": xt,
            "xn": xn,
            "m": (wq64 @ wk64.T).astype(np.float32),
            "g": (wv64 @ wp64).astype(np.float32),
        })

    nc = _get_nc()
    try:
        res = run_bass_kernel_spmd(nc, in_maps, list(range(H)), trace=trace)
    except Exception:
        res = run_bass_kernel_spmd(nc, in_maps, list(range(H)), trace=trace)
    acc = res.results[0]["out"].astype(np.float32, copy=True)
    for h in range(1, H):
        acc += res.results[h]["out"]
    out = acc.reshape(B, S, D) + bp[None, None, :]
    return out.astype(np.float32), res


def kernel(**inputs):
    out, _ = _run(inputs, trace=False)
    return out


# revision 21
# speedup vs baseline: 1.0060x; 1.0060x over previous
"""Multi-head attention (B=4, S=2048, D=512, H=8, inner=512) on 8 trn2 cores.

Sharding: tensor-parallel over heads. Core h computes head h end-to-end;
the host sums the 8 partial output projections.

Because inner == D, the per-head algebra factors so both the k and v
projections vanish from the device program:
  scores = (x Wq)(x Wk)^T = x (Wq Wk^T) x^T      M = Wq Wk^T  (host, fp64)
  out_h  = (P (x Wv)) Wp_h = (P x)(Wv Wp_h)      G = Wv Wp_h  (host, fp64)
so the device only computes q' = x M, scoresT = x q'^T, z = P x, z G.

Device layout (matmuls in float32r: full PE rate, ~1.3e-4 matmul error):
  xt [D, B*S] and xn [B*S, D] are host-prepared so both the d-contraction
  (scores/q') and t-contraction (z = P x) have their operands partition-
  aligned. scoresT tiles are [t_block, sq] so softmax's key-axis sum is a
  partition reduction: P accumulates on the vector engine, and 4 tiny
  N=1 fp32 matmuls against a ones column give per-query sums in column
  layout for the reciprocal. exp needs no max-subtraction (|scores| <~ 35
  for this data, far from fp32 overflow). Normalization is deferred to
  the output projection, applied as a per-partition scalar on the
  PSUM->SBUF move.

The bias inputs (bq/bk/bv/bp) are structurally zero for this problem
(spec fill=zeros); bp is added on host, and a host fallback covers the
(per-spec impossible) nonzero q/k/v bias case.
"""

import numpy as np

import concourse.mybir as mybir
import concourse.tile as tile
from concourse import bacc
from concourse.bass_utils import run_bass_kernel_spmd

F32 = mybir.dt.float32
F32R = mybir.dt.float32r

B, S, D, H = 4, 2048, 512, 8
E = D  # per-head inner size
NKD = D // 128   # contraction chunks over D
NW = S // 512    # query windows per batch
NT = S // 128    # key blocks per batch
ISQRT_E = 1.0 / float(np.sqrt(E))

_CACHE = {}


def _build():
    nc = bacc.Bacc("TRN2", target_bir_lowering=False, debug=False, num_devices=8)

    xt_ext = nc.dram_tensor("xt", [D, B * S], F32R, kind="ExternalInput")
    xn_ext = nc.dram_tensor("xn", [B * S, D], F32R, kind="ExternalInput")
    m_ext = nc.dram_tensor("m", [D, D], F32R, kind="ExternalInput")
    g_ext = nc.dram_tensor("g", [D, D], F32R, kind="ExternalInput")
    out_ext = nc.dram_tensor("out", [B * S, D], F32, kind="ExternalOutput")
    dbg_ext = nc.dram_tensor("dbg", [1, 64], F32, kind="ExternalOutput")

    with tile.TileContext(nc) as tc:
        with (
            tc.tile_pool(name="wpool", bufs=1) as wpool,
            tc.tile_pool(name="xpool", bufs=2) as xpool,
            tc.tile_pool(name="actpool", bufs=2) as actpool,
            tc.tile_pool(name="qtpool", bufs=2) as qtpool,
            tc.tile_pool(name="ppool", bufs=3) as ppool,
            tc.tile_pool(name="otpool", bufs=1) as otpool,
            tc.tile_pool(name="opool", bufs=3) as opool,
            tc.tile_pool(name="rpool", bufs=1) as rpool,
            tc.tile_pool(name="mm_ps", bufs=4, space="PSUM") as mm_ps,
            tc.tile_pool(name="o_ps", bufs=1, space="PSUM") as o_ps_pool,
        ):
            # dummy matmuls during the initial DMA window lift the PE's HAM
            # clock gate to 2.4GHz before the first real matmul arrives
            warm_sb = wpool.tile([128, 128], F32)
            nc.vector.memset(warm_sb[:], 0.0)
            warm_ps = mm_ps.tile([128, 64], F32, name="warmps", tag="mm")
            for _ in range(24):
                nc.tensor.matmul(warm_ps[:], warm_sb[:, 0:128], warm_sb[:, 0:64],
                                 start=True, stop=True)
            warm_out = wpool.tile([1, 64], F32)
            nc.vector.tensor_copy(warm_out[:], warm_ps[0:1, :])
            nc.sync.dma_start(out=dbg_ext[:], in_=warm_out[:])

            m_sb = wpool.tile([128, NKD, D], F32R)
            g_sb = wpool.tile([128, NKD, D], F32R)
            for k in range(NKD):
                nc.sync.dma_start(out=m_sb[:, k, :],
                                  in_=m_ext[k * 128:(k + 1) * 128, :])

            ones_f32 = wpool.tile([128, 1], F32)
            nc.vector.memset(ones_f32[:], 1.0)

            # x in natural [t, d] layout is the stationary operand of
            # z = P x -- pure data movement, no projection matmuls. Loaded
            # one batch ahead so the descriptors clear the sync queue
            # before that batch's output DMAs pile in behind them.
            xn_tiles = {}

            def load_xn(bb):
                t_sb = actpool.tile([128, NT, D], F32R, name=f"xn{bb}", tag="v")
                for t in range(NT):
                    r0 = bb * S + t * 128
                    nc.sync.dma_start(out=t_sb[:, t, :], in_=xn_ext[r0:r0 + 128, :])
                xn_tiles[bb] = t_sb

            # xt, transposed x, feeds q' and the scores stationary operand;
            # descriptors go out on the idle gpsimd queue so they issue in
            # parallel with xn/m on the sync queue
            xt_tiles = {}

            def load_xt(bb):
                t_sb = xpool.tile([128, NKD, S], F32R, name=f"xt{bb}", tag="xt")
                for w in range(NW):
                    for k in range(NKD):
                        nc.gpsimd.dma_start(
                            out=t_sb[:, k, w * 512:(w + 1) * 512],
                            in_=xt_ext[k * 128:(k + 1) * 128,
                                       bb * S + w * 512:bb * S + (w + 1) * 512],
                        )
                xt_tiles[bb] = t_sb

            def emit_qt(bb, w):
                wsl = slice(w * 512, (w + 1) * 512)
                x_sb = xt_tiles[bb]
                qt_sb = qtpool.tile([128, NKD, 512], F32R, name="qtw", tag="qt")
                for me in range(NKD):
                    msl = slice(me * 128, (me + 1) * 128)
                    ps = mm_ps.tile([128, 512], F32, name="mmps", tag="mm")
                    for k in range(NKD):
                        nc.tensor.matmul(
                            ps[:], m_sb[:, k, msl], x_sb[:, k, wsl],
                            start=(k == 0), stop=(k == NKD - 1),
                        )
                    nc.vector.tensor_copy(qt_sb[:, me, :], ps[:])
                return qt_sb

            load_xn(0)
            load_xt(0)
            qt_sb = None
            for b in range(B):
                if b == 0:
                    # g's first use is the first output projection, ~40us in
                    for k in range(NKD):
                        nc.gpsimd.dma_start(out=g_sb[:, k, :],
                                            in_=g_ext[k * 128:(k + 1) * 128, :])
                if b + 1 < B:
                    load_xn(b + 1)
                    load_xt(b + 1)
                xn_sb = xn_tiles.pop(b)
                xt_sb = xt_tiles[b]

                if qt_sb is None:
                    qt_sb = emit_qt(0, 0)
                for w in range(NW):
                    o_ps = o_ps_pool.tile([128, NKD, 512], F32, name="ops", tag="ops")
                    p_acc = rpool.tile([128, 512], F32, name="pacc", tag="pacc")

                    # software-pipelined by one t-block: scores(t+1) is
                    # emitted before z(t) so the PE never stalls on exp(t)
                    s_tiles = {}
                    s_tiles[0] = mm_ps.tile([128, 512], F32, name="mmps", tag="mm")
                    for k in range(NKD):
                        nc.tensor.matmul(
                            s_tiles[0][:], xt_sb[:, k, 0:128], qt_sb[:, k, :],
                            start=(k == 0), stop=(k == NKD - 1),
                        )
                    for t in range(NT):
                        if t + 1 < NT:
                            tsl = slice((t + 1) * 128, (t + 2) * 128)
                            nxt = mm_ps.tile([128, 512], F32, name="mmps", tag="mm")
                            for k in range(NKD):
                                nc.tensor.matmul(
                                    nxt[:], xt_sb[:, k, tsl], qt_sb[:, k, :],
                                    start=(k == 0), stop=(k == NKD - 1),
                                )
                            s_tiles[t + 1] = nxt
                        p_sb = ppool.tile([128, 512], F32R, name="ptile", tag="p")
                        nc.scalar.activation(
                            p_sb[:], s_tiles.pop(t)[:],
                            mybir.ActivationFunctionType.Exp, scale=ISQRT_E,
                        )
                        # rowsum accumulates on the vector engine instead of
                        # burning a PE matmul per t-block
                        p_in = p_sb[:].bitcast(F32)
                        if t == 0:
                            nc.vector.tensor_copy(p_acc[:], p_in)
                        else:
                            nc.vector.tensor_add(p_acc[:], p_acc[:], p_in)
                        for me in range(NKD):
                            msl = slice(me * 128, (me + 1) * 128)
                            nc.tensor.matmul(
                                o_ps[:, me, :], xn_sb[:, t, msl], p_sb[:],
                                start=(t == 0), stop=(t == NT - 1),
                                skip_group_check=True,
                            )

                    # scalar engine moves z out of PSUM (frees banks for the
                    # next window while the vector engine handles rowsums)
                    zt_sb = otpool.tile([128, NKD, 512], F32R, name="zt", tag="ot")
                    for me in range(NKD):
                        nc.scalar.copy(zt_sb[:, me, :], o_ps[:, me, :])

                    # prefetch the next (batch, window)'s q' -- across batch
                    # seams too -- so the PE stays busy while the
                    # normalization chain below runs on DVE/ACT
                    if w + 1 < NW:
                        qt_next = emit_qt(b, w + 1)
                    elif b + 1 < B:
                        qt_next = emit_qt(b + 1, 0)
                    else:
                        qt_next = None

                    # per-query rowsums straight into column layout:
                    # rtp[:, j] = p_acc[:, j-block].T @ ones -- tiny N=1 fp32
                    # matmuls (fp32r forbids N=1; 4 cyc/row x 1 row is free)
                    rtp = mm_ps.tile([128, 4], F32, name="rtp", tag="mm")
                    for j in range(4):
                        nc.tensor.matmul(
                            rtp[:, j:j + 1],
                            p_acc[:, j * 128:(j + 1) * 128], ones_f32[:],
                            start=True, stop=True,
                        )
                    rraw = rpool.tile([128, 4], F32, name="rraw", tag="rraw")
                    nc.vector.tensor_copy(rraw[:], rtp[:])
                    rcol = rpool.tile([128, 4], F32, name="rcol", tag="rc")
                    nc.vector.reciprocal(rcol[:], rraw[:])

                    # output projection for this window; normalization is the
                    # per-partition scalar multiply on the PSUM->SBUF move
                    for j in range(4):
                        jsl = slice(j * 128, (j + 1) * 128)
                        ps = mm_ps.tile([128, 512], F32, name="mmps", tag="mm")
                        for me in range(NKD):
                            nc.tensor.matmul(
                                ps[:], zt_sb[:, me, jsl], g_sb[:, me, :],
                                start=(me == 0), stop=(me == NKD - 1),
                            )
                        po_sb = opool.tile([128, 512], F32, name="po", tag="po")
                        nc.vector.tensor_scalar(
                            po_sb[:], ps[:], rcol[:, j:j + 1], None,
                            mybir.AluOpType.mult,
                        )
                        # output descriptors ride the scalar queue: the sync
                        # queue's xn slot-waits at batch seams must not block
                        # them, or po slots back up into the PE's psum pool
                        row0 = b * S + w * 512 + j * 128
                        nc.scalar.dma_start(
                            out=out_ext[row0:row0 + 128, :], in_=po_sb[:]
                        )
                    qt_sb = qt_next

    nc.compile()
    return nc


def _get_nc():
    if "nc" not in _CACHE:
        _CACHE["nc"] = _build()
    return _CACHE["nc"]


def _numpy_fallback(emb, Wq, bq, Wk, bk, Wv, bv, Wp, bp):
    x = emb.astype(np.float64)
    out = np.zeros((B, S, D), dtype=np.float64)
    for h in range(H):
        q = x @ Wq[h].astype(np.float64) + bq[h]
        k = x @ Wk[h].astype(np.float64) + bk[h]
        v = x @ Wv[h].astype(np.float64) + bv[h]
        for b in range(B):
            sc = (q[b] @ k[b].T) / np.sqrt(E)
            sc -= sc.max(axis=1, keepdims=True)
            p = np.exp(sc)
            p /= p.sum(axis=1, keepdims=True)
            out[b] += (p @ v[b]) @ Wp[h * E:(h + 1) * E].astype(np.float64)
    return (out + bp).astype(np.float32)


def _run(inputs, trace=False):
    emb = np.ascontiguousarray(inputs["emb_input"], dtype=np.float32)
    Wq = np.ascontiguousarray(inputs["Wq"], dtype=np.float32)
    Wk = np.ascontiguousarray(inputs["Wk"], dtype=np.float32)
    Wv = np.ascontiguousarray(inputs["Wv"], dtype=np.float32)
    Wp = np.ascontiguousarray(inputs["Wp"], dtype=np.float32)
    bq = np.asarray(inputs["bq"], dtype=np.float32)
    bk = np.asarray(inputs["bk"], dtype=np.float32)
    bv = np.asarray(inputs["bv"], dtype=np.float32)
    bp = np.asarray(inputs["bp"], dtype=np.float32)

    if np.any(bq) or np.any(bk) or np.any(bv):
        # the device program folds Wq/Wk and Wv/Wp together, which assumes
        # the q/k/v biases are structurally zero (problem spec fill=zeros);
        # anything else falls back to host math
        return _numpy_fallback(emb, Wq, bq, Wk, bk, Wv, bv, Wp, bp), None

    xt = np.ascontiguousarray(emb.transpose(2, 0, 1).reshape(D, B * S))
    xn = emb.reshape(B * S, D)
    in_maps = []
    for h in range(H):
        wq64 = Wq[h].astype(np.float64)
        wk64 = Wk[h].astype(np.float64)
        wv64 = Wv[h].astype(np.float64)
        wp64 = Wp[h * E:(h + 1) * E, :].astype(np.float64)
        in_maps.append({
            "xt": xt,
            "xn": xn,
            "m": (wq64 @ wk64.T).astype(np.float32),
            "g": (wv64 @ wp64).astype(np.float32),
        })

    nc = _get_nc()
    try:
        res = run_bass_kernel_spmd(nc, in_maps, list(range(H)), trace=trace)
    except Exception:
        res = run_bass_kernel_spmd(nc, in_maps, list(range(H)), trace=trace)
    acc = res.results[0]["out"].astype(np.float32, copy=True)
    for h in range(1, H):
        acc += res.results[h]["out"]
    out = acc.reshape(B, S, D) + bp[None, None, :]
    return out.astype(np.float32), res


def kernel(**inputs):
    out, _ = _run(inputs, trace=False)
    return out


# revision 23
# speedup vs baseline: 1.0159x; 1.0099x over previous
"""Multi-head attention (B=4, S=2048, D=512, H=8, inner=512) on 8 trn2 cores.

Sharding: tensor-parallel over heads. Core h computes head h end-to-end;
the host sums the 8 partial output projections.

Because inner == D, the per-head algebra factors so both the k and v
projections vanish from the device program:
  scores = (x Wq)(x Wk)^T = x (Wq Wk^T) x^T      M = Wq Wk^T  (host, fp64)
  out_h  = (P (x Wv)) Wp_h = (P x)(Wv Wp_h)      G = Wv Wp_h  (host, fp64)
so the device only computes q' = x M, scoresT = x q'^T, z = P x, z G.

Device layout (matmuls in float32r: full PE rate, ~1.3e-4 matmul error):
  xt [D, B*S] and xn [B*S, D] are host-prepared so both the d-contraction
  (scores/q') and t-contraction (z = P x) have their operands partition-
  aligned. scoresT tiles are [t_block, sq] so softmax's key-axis sum is a
  partition reduction: P accumulates on the vector engine, and 4 tiny
  N=1 fp32 matmuls against a ones column give per-query sums in column
  layout for the reciprocal. exp needs no max-subtraction (|scores| <~ 35
  for this data, far from fp32 overflow). Normalization is deferred to
  the output projection, applied as a per-partition scalar on the
  PSUM->SBUF move.

The bias inputs (bq/bk/bv/bp) are structurally zero for this problem
(spec fill=zeros); bp is added on host, and a host fallback covers the
(per-spec impossible) nonzero q/k/v bias case.
"""

import numpy as np

import concourse.mybir as mybir
import concourse.tile as tile
from concourse import bacc
from concourse.bass_utils import run_bass_kernel_spmd

F32 = mybir.dt.float32
F32R = mybir.dt.float32r

B, S, D, H = 4, 2048, 512, 8
E = D  # per-head inner size
NKD = D // 128   # contraction chunks over D
NW = S // 512    # query windows per batch
NT = S // 128    # key blocks per batch
ISQRT_E = 1.0 / float(np.sqrt(E))

_CACHE = {}


def _build():
    nc = bacc.Bacc("TRN2", target_bir_lowering=False, debug=False, num_devices=8)

    xt_ext = nc.dram_tensor("xt", [D, B * S], F32R, kind="ExternalInput")
    xn_ext = nc.dram_tensor("xn", [B * S, D], F32R, kind="ExternalInput")
    m_ext = nc.dram_tensor("m", [D, D], F32R, kind="ExternalInput")
    g_ext = nc.dram_tensor("g", [D, D], F32R, kind="ExternalInput")
    out_ext = nc.dram_tensor("out", [B * S, D], F32, kind="ExternalOutput")
    dbg_ext = nc.dram_tensor("dbg", [1, 64], F32, kind="ExternalOutput")

    with tile.TileContext(nc) as tc:
        with (
            tc.tile_pool(name="wpool", bufs=1) as wpool,
            tc.tile_pool(name="xpool", bufs=2) as xpool,
            tc.tile_pool(name="actpool", bufs=2) as actpool,
            tc.tile_pool(name="qtpool", bufs=2) as qtpool,
            tc.tile_pool(name="ppool", bufs=3) as ppool,
            tc.tile_pool(name="otpool", bufs=1) as otpool,
            tc.tile_pool(name="opool", bufs=3) as opool,
            tc.tile_pool(name="rpool", bufs=1) as rpool,
            tc.tile_pool(name="mm_ps", bufs=4, space="PSUM") as mm_ps,
            tc.tile_pool(name="o_ps", bufs=1, space="PSUM") as o_ps_pool,
        ):
            # dummy matmuls during the initial DMA window lift the PE's HAM
            # clock gate to 2.4GHz before the first real matmul arrives
            warm_sb = wpool.tile([128, 128], F32)
            nc.vector.memset(warm_sb[:], 0.0)
            warm_ps = mm_ps.tile([128, 64], F32, name="warmps", tag="mm")
            for _ in range(24):
                nc.tensor.matmul(warm_ps[:], warm_sb[:, 0:128], warm_sb[:, 0:64],
                                 start=True, stop=True)
            warm_out = wpool.tile([1, 64], F32)
            nc.vector.tensor_copy(warm_out[:], warm_ps[0:1, :])
            nc.sync.dma_start(out=dbg_ext[:], in_=warm_out[:])

            m_sb = wpool.tile([128, NKD, D], F32R)
            g_sb = wpool.tile([128, NKD, D], F32R)
            for k in range(NKD):
                nc.sync.dma_start(out=m_sb[:, k, :],
                                  in_=m_ext[k * 128:(k + 1) * 128, :])

            ones_f32 = wpool.tile([128, 1], F32)
            nc.vector.memset(ones_f32[:], 1.0)

            # x in natural [t, d] layout is the stationary operand of
            # z = P x -- pure data movement, no projection matmuls. Loaded
            # one batch ahead so the descriptors clear the sync queue
            # before that batch's output DMAs pile in behind them.
            xn_tiles = {}

            def load_xn(bb):
                # batch 0 rides the sync queue (needed immediately, no slot
                # wait); later batches go on gpsimd where their slot-waits at
                # batch seams cannot block the output descriptors on sync
                eng = nc.sync if bb == 0 else nc.gpsimd
                t_sb = actpool.tile([128, NT, D], F32R, name=f"xn{bb}", tag="v")
                for t in range(NT):
                    r0 = bb * S + t * 128
                    eng.dma_start(out=t_sb[:, t, :], in_=xn_ext[r0:r0 + 128, :])
                xn_tiles[bb] = t_sb

            # xt, transposed x, feeds q' and the scores stationary operand;
            # descriptors go out on the idle gpsimd queue so they issue in
            # parallel with xn/m on the sync queue
            xt_tiles = {}

            def load_xt(bb):
                t_sb = xpool.tile([128, NKD, S], F32R, name=f"xt{bb}", tag="xt")
                for w in range(NW):
                    for k in range(NKD):
                        nc.gpsimd.dma_start(
                            out=t_sb[:, k, w * 512:(w + 1) * 512],
                            in_=xt_ext[k * 128:(k + 1) * 128,
                                       bb * S + w * 512:bb * S + (w + 1) * 512],
                        )
                xt_tiles[bb] = t_sb

            def emit_qt(bb, w):
                wsl = slice(w * 512, (w + 1) * 512)
                x_sb = xt_tiles[bb]
                qt_sb = qtpool.tile([128, NKD, 512], F32R, name="qtw", tag="qt")
                for me in range(NKD):
                    msl = slice(me * 128, (me + 1) * 128)
                    ps = mm_ps.tile([128, 512], F32, name="mmps", tag="mm")
                    for k in range(NKD):
                        nc.tensor.matmul(
                            ps[:], m_sb[:, k, msl], x_sb[:, k, wsl],
                            start=(k == 0), stop=(k == NKD - 1),
                        )
                    nc.vector.tensor_copy(qt_sb[:, me, :], ps[:])
                return qt_sb

            load_xn(0)
            load_xt(0)
            qt_sb = None
            for b in range(B):
                if b == 0:
                    # g's first use is the first output projection, ~40us in
                    for k in range(NKD):
                        nc.gpsimd.dma_start(out=g_sb[:, k, :],
                                            in_=g_ext[k * 128:(k + 1) * 128, :])
                if b + 1 < B:
                    load_xn(b + 1)
                    load_xt(b + 1)
                xn_sb = xn_tiles.pop(b)
                xt_sb = xt_tiles[b]

                if qt_sb is None:
                    qt_sb = emit_qt(0, 0)
                for w in range(NW):
                    o_ps = o_ps_pool.tile([128, NKD, 512], F32, name="ops", tag="ops")
                    p_acc = rpool.tile([128, 512], F32, name="pacc", tag="pacc")

                    # software-pipelined by one t-block: scores(t+1) is
                    # emitted before z(t) so the PE never stalls on exp(t)
                    s_tiles = {}
                    s_tiles[0] = mm_ps.tile([128, 512], F32, name="mmps", tag="mm")
                    for k in range(NKD):
                        nc.tensor.matmul(
                            s_tiles[0][:], xt_sb[:, k, 0:128], qt_sb[:, k, :],
                            start=(k == 0), stop=(k == NKD - 1),
                        )
                    for t in range(NT):
                        if t + 1 < NT:
                            tsl = slice((t + 1) * 128, (t + 2) * 128)
                            nxt = mm_ps.tile([128, 512], F32, name="mmps", tag="mm")
                            for k in range(NKD):
                                nc.tensor.matmul(
                                    nxt[:], xt_sb[:, k, tsl], qt_sb[:, k, :],
                                    start=(k == 0), stop=(k == NKD - 1),
                                )
                            s_tiles[t + 1] = nxt
                        p_sb = ppool.tile([128, 512], F32R, name="ptile", tag="p")
                        nc.scalar.activation(
                            p_sb[:], s_tiles.pop(t)[:],
                            mybir.ActivationFunctionType.Exp, scale=ISQRT_E,
                        )
                        # rowsum accumulates on the vector engine instead of
                        # burning a PE matmul per t-block
                        p_in = p_sb[:].bitcast(F32)
                        if t == 0:
                            nc.vector.tensor_copy(p_acc[:], p_in)
                        else:
                            nc.vector.tensor_add(p_acc[:], p_acc[:], p_in)
                        for me in range(NKD):
                            msl = slice(me * 128, (me + 1) * 128)
                            nc.tensor.matmul(
                                o_ps[:, me, :], xn_sb[:, t, msl], p_sb[:],
                                start=(t == 0), stop=(t == NT - 1),
                                skip_group_check=True,
                            )

                    # scalar engine moves z out of PSUM (frees banks for the
                    # next window while the vector engine handles rowsums)
                    zt_sb = otpool.tile([128, NKD, 512], F32R, name="zt", tag="ot")
                    for me in range(NKD):
                        nc.scalar.copy(zt_sb[:, me, :], o_ps[:, me, :])

                    # prefetch the next (batch, window)'s q' -- across batch
                    # seams too -- so the PE stays busy while the
                    # normalization chain below runs on DVE/ACT
                    if w + 1 < NW:
                        qt_next = emit_qt(b, w + 1)
                    elif b + 1 < B:
                        qt_next = emit_qt(b + 1, 0)
                    else:
                        qt_next = None

                    # per-query rowsums straight into column layout:
                    # rtp[:, j] = p_acc[:, j-block].T @ ones -- tiny N=1 fp32
                    # matmuls (fp32r forbids N=1; 4 cyc/row x 1 row is free)
                    rtp = mm_ps.tile([128, 4], F32, name="rtp", tag="mm")
                    for j in range(4):
                        nc.tensor.matmul(
                            rtp[:, j:j + 1],
                            p_acc[:, j * 128:(j + 1) * 128], ones_f32[:],
                            start=True, stop=True,
                        )
                    rraw = rpool.tile([128, 4], F32, name="rraw", tag="rraw")
                    nc.vector.tensor_copy(rraw[:], rtp[:])
                    rcol = rpool.tile([128, 4], F32, name="rcol", tag="rc")
                    nc.vector.reciprocal(rcol[:], rraw[:])

                    # output projection for this window; its psum lives in the
                    # o_ps pool slot (freed above by the zt copies) so the
                    # mm pool's scores/qt slots never wait on the slower
                    # normalization drain below
                    proj_ps = o_ps_pool.tile([128, NKD, 512], F32,
                                             name="projps", tag="ops")
                    for j in range(4):
                        jsl = slice(j * 128, (j + 1) * 128)
                        for me in range(NKD):
                            nc.tensor.matmul(
                                proj_ps[:, j, :], zt_sb[:, me, jsl], g_sb[:, me, :],
                                start=(me == 0), stop=(me == NKD - 1),
                            )
                        po_sb = opool.tile([128, 512], F32, name="po", tag="po")
                        # normalization: per-partition scalar on the
                        # PSUM->SBUF move
                        nc.vector.tensor_scalar(
                            po_sb[:], proj_ps[:, j, :], rcol[:, j:j + 1], None,
                            mybir.AluOpType.mult,
                        )
                        row0 = b * S + w * 512 + j * 128
                        nc.sync.dma_start(
                            out=out_ext[row0:row0 + 128, :], in_=po_sb[:]
                        )
                    qt_sb = qt_next

    nc.compile()
    return nc


def _get_nc():
    if "nc" not in _CACHE:
        _CACHE["nc"] = _build()
    return _CACHE["nc"]


def _numpy_fallback(emb, Wq, bq, Wk, bk, Wv, bv, Wp, bp):
    x = emb.astype(np.float64)
    out = np.zeros((B, S, D), dtype=np.float64)
    for h in range(H):
        q = x @ Wq[h].astype(np.float64) + bq[h]
        k = x @ Wk[h].astype(np.float64) + bk[h]
        v = x @ Wv[h].astype(np.float64) + bv[h]
        for b in range(B):
            sc = (q[b] @ k[b].T) / np.sqrt(E)
            sc -= sc.max(axis=1, keepdims=True)
            p = np.exp(sc)
            p /= p.sum(axis=1, keepdims=True)
            out[b] += (p @ v[b]) @ Wp[h * E:(h + 1) * E].astype(np.float64)
    return (out + bp).astype(np.float32)


def _run(inputs, trace=False):
    emb = np.ascontiguousarray(inputs["emb_input"], dtype=np.float32)
    Wq = np.ascontiguousarray(inputs["Wq"], dtype=np.float32)
    Wk = np.ascontiguousarray(inputs["Wk"], dtype=np.float32)
    Wv = np.ascontiguousarray(inputs["Wv"], dtype=np.float32)
    Wp = np.ascontiguousarray(inputs["Wp"], dtype=np.float32)
    bq = np.asarray(inputs["bq"], dtype=np.float32)
    bk = np.asarray(inputs["bk"], dtype=np.float32)
    bv = np.asarray(inputs["bv"], dtype=np.float32)
    bp = np.asarray(inputs["bp"], dtype=np.float32)

    if np.any(bq) or np.any(bk) or np.any(bv):
        # the device program folds Wq/Wk and Wv/Wp together, which assumes
        # the q/k/v biases are structurally zero (problem spec fill=zeros);
        # anything else falls back to host math
        return _numpy_fallback(emb, Wq, bq, Wk, bk, Wv, bv, Wp, bp), None

    xt = np.ascontiguousarray(emb.transpose(2, 0, 1).reshape(D, B * S))
    xn = emb.reshape(B * S, D)
    in_maps = []
    for h in range(H):
        wq64 = Wq[h].astype(np.float64)
        wk64 = Wk[h].astype(np.float64)
        wv64 = Wv[h].astype(np.float64)
        wp64 = Wp[h * E:(h + 1) * E, :].astype(np.float64)
        in_maps.append({
            "xt": xt,
            "xn": xn,
            "m": (wq64 @ wk64.T).astype(np.float32),
            "g": (wv64 @ wp64).astype(np.float32),
        })

    nc = _get_nc()
    try:
        res = run_bass_kernel_spmd(nc, in_maps, list(range(H)), trace=trace)
    except Exception:
        res = run_bass_kernel_spmd(nc, in_maps, list(range(H)), trace=trace)
    acc = res.results[0]["out"].astype(np.float32, copy=True)
    for h in range(1, H):
        acc += res.results[h]["out"]
    out = acc.reshape(B, S, D) + bp[None, None, :]
    return out.astype(np.float32), res


def kernel(**inputs):
    out, _ = _run(inputs, trace=False)
    return out


# revision 24
# speedup vs baseline: 1.0229x; 1.0069x over previous
"""Multi-head attention (B=4, S=2048, D=512, H=8, inner=512) on 8 trn2 cores.

Sharding: tensor-parallel over heads. Core h computes head h end-to-end;
the host sums the 8 partial output projections.

Because inner == D, the per-head algebra factors so both the k and v
projections vanish from the device program:
  scores = (x Wq)(x Wk)^T = x (Wq Wk^T) x^T      M = Wq Wk^T  (host, fp64)
  out_h  = (P (x Wv)) Wp_h = (P x)(Wv Wp_h)      G = Wv Wp_h  (host, fp64)
so the device only computes q' = x M, scoresT = x q'^T, z = P x, z G.

Device layout (matmuls in float32r: full PE rate, ~1.3e-4 matmul error):
  xt [D, B*S] and xn [B*S, D] are host-prepared so both the d-contraction
  (scores/q') and t-contraction (z = P x) have their operands partition-
  aligned. scoresT tiles are [t_block, sq] so softmax's key-axis sum is a
  partition reduction: P accumulates on the vector engine, and 4 tiny
  N=1 fp32 matmuls against a ones column give per-query sums in column
  layout for the reciprocal. exp needs no max-subtraction (|scores| <~ 35
  for this data, far from fp32 overflow). Normalization is deferred to
  the output projection, applied as a per-partition scalar on the
  PSUM->SBUF move.

The bias inputs (bq/bk/bv/bp) are structurally zero for this problem
(spec fill=zeros); bp is added on host, and a host fallback covers the
(per-spec impossible) nonzero q/k/v bias case.
"""

import numpy as np

import concourse.mybir as mybir
import concourse.tile as tile
from concourse import bacc
from concourse.bass_utils import run_bass_kernel_spmd

F32 = mybir.dt.float32
F32R = mybir.dt.float32r

B, S, D, H = 4, 2048, 512, 8
E = D  # per-head inner size
NKD = D // 128   # contraction chunks over D
NW = S // 512    # query windows per batch
NT = S // 128    # key blocks per batch
ISQRT_E = 1.0 / float(np.sqrt(E))

_CACHE = {}


def _build():
    nc = bacc.Bacc("TRN2", target_bir_lowering=False, debug=False, num_devices=8)

    xt_ext = nc.dram_tensor("xt", [D, B * S], F32R, kind="ExternalInput")
    xn_ext = nc.dram_tensor("xn", [B * S, D], F32R, kind="ExternalInput")
    m_ext = nc.dram_tensor("m", [D, D], F32R, kind="ExternalInput")
    g_ext = nc.dram_tensor("g", [D, D], F32R, kind="ExternalInput")
    out_ext = nc.dram_tensor("out", [B * S, D], F32, kind="ExternalOutput")
    dbg_ext = nc.dram_tensor("dbg", [1, 64], F32, kind="ExternalOutput")

    with tile.TileContext(nc) as tc:
        with (
            tc.tile_pool(name="wpool", bufs=1) as wpool,
            tc.tile_pool(name="xpool", bufs=2) as xpool,
            tc.tile_pool(name="actpool", bufs=2) as actpool,
            tc.tile_pool(name="qtpool", bufs=2) as qtpool,
            tc.tile_pool(name="ppool", bufs=4) as ppool,
            tc.tile_pool(name="otpool", bufs=1) as otpool,
            tc.tile_pool(name="opool", bufs=3) as opool,
            tc.tile_pool(name="rpool", bufs=1) as rpool,
            tc.tile_pool(name="mm_ps", bufs=4, space="PSUM") as mm_ps,
            tc.tile_pool(name="o_ps", bufs=1, space="PSUM") as o_ps_pool,
        ):
            # dummy matmuls during the initial DMA window lift the PE's HAM
            # clock gate to 2.4GHz before the first real matmul arrives
            warm_sb = wpool.tile([128, 128], F32)
            nc.vector.memset(warm_sb[:], 0.0)
            warm_ps = mm_ps.tile([128, 64], F32, name="warmps", tag="mm")
            for _ in range(24):
                nc.tensor.matmul(warm_ps[:], warm_sb[:, 0:128], warm_sb[:, 0:64],
                                 start=True, stop=True)
            warm_out = wpool.tile([1, 64], F32)
            nc.vector.tensor_copy(warm_out[:], warm_ps[0:1, :])
            nc.sync.dma_start(out=dbg_ext[:], in_=warm_out[:])

            m_sb = wpool.tile([128, NKD, D], F32R)
            g_sb = wpool.tile([128, NKD, D], F32R)
            for k in range(NKD):
                nc.sync.dma_start(out=m_sb[:, k, :],
                                  in_=m_ext[k * 128:(k + 1) * 128, :])

            ones_f32 = wpool.tile([128, 1], F32)
            nc.vector.memset(ones_f32[:], 1.0)

            # x in natural [t, d] layout is the stationary operand of
            # z = P x -- pure data movement, no projection matmuls. Loaded
            # one batch ahead so the descriptors clear the sync queue
            # before that batch's output DMAs pile in behind them.
            xn_tiles = {}

            def load_xn(bb):
                # batch 0 rides the sync queue (needed immediately, no slot
                # wait); later batches go on gpsimd where their slot-waits at
                # batch seams cannot block the output descriptors on sync
                eng = nc.sync if bb == 0 else nc.gpsimd
                t_sb = actpool.tile([128, NT, D], F32R, name=f"xn{bb}", tag="v")
                for t in range(NT):
                    r0 = bb * S + t * 128
                    eng.dma_start(out=t_sb[:, t, :], in_=xn_ext[r0:r0 + 128, :])
                xn_tiles[bb] = t_sb

            # xt, transposed x, feeds q' and the scores stationary operand;
            # descriptors go out on the idle gpsimd queue so they issue in
            # parallel with xn/m on the sync queue
            xt_tiles = {}

            def load_xt(bb):
                t_sb = xpool.tile([128, NKD, S], F32R, name=f"xt{bb}", tag="xt")
                for w in range(NW):
                    for k in range(NKD):
                        nc.gpsimd.dma_start(
                            out=t_sb[:, k, w * 512:(w + 1) * 512],
                            in_=xt_ext[k * 128:(k + 1) * 128,
                                       bb * S + w * 512:bb * S + (w + 1) * 512],
                        )
                xt_tiles[bb] = t_sb

            def emit_qt(bb, w):
                wsl = slice(w * 512, (w + 1) * 512)
                x_sb = xt_tiles[bb]
                qt_sb = qtpool.tile([128, NKD, 512], F32R, name="qtw", tag="qt")
                for me in range(NKD):
                    msl = slice(me * 128, (me + 1) * 128)
                    ps = mm_ps.tile([128, 512], F32, name="mmps", tag="mm")
                    for k in range(NKD):
                        nc.tensor.matmul(
                            ps[:], m_sb[:, k, msl], x_sb[:, k, wsl],
                            start=(k == 0), stop=(k == NKD - 1),
                        )
                    nc.vector.tensor_copy(qt_sb[:, me, :], ps[:])
                return qt_sb

            load_xn(0)
            load_xt(0)
            qt_sb = None
            for b in range(B):
                if b == 0:
                    # g's first use is the first output projection, ~40us in
                    for k in range(NKD):
                        nc.gpsimd.dma_start(out=g_sb[:, k, :],
                                            in_=g_ext[k * 128:(k + 1) * 128, :])
                if b + 1 < B:
                    load_xn(b + 1)
                    load_xt(b + 1)
                xn_sb = xn_tiles.pop(b)
                xt_sb = xt_tiles[b]

                if qt_sb is None:
                    qt_sb = emit_qt(0, 0)
                for w in range(NW):
                    o_ps = o_ps_pool.tile([128, NKD, 512], F32, name="ops", tag="ops")
                    p_acc = rpool.tile([128, 512], F32, name="pacc", tag="pacc")

                    # software-pipelined two t-blocks ahead: scores(t+1) and
                    # scores(t+2) are emitted before z(t) so the PE never
                    # stalls on exp(t) even across group boundaries
                    s_tiles = {}

                    def emit_scores(tt):
                        tsl = slice(tt * 128, (tt + 1) * 128)
                        ps = mm_ps.tile([128, 512], F32, name="mmps", tag="mm")
                        for k in range(NKD):
                            nc.tensor.matmul(
                                ps[:], xt_sb[:, k, tsl], qt_sb[:, k, :],
                                start=(k == 0), stop=(k == NKD - 1),
                            )
                        s_tiles[tt] = ps

                    emit_scores(0)
                    emit_scores(1)
                    for t in range(NT):
                        if t + 2 < NT:
                            emit_scores(t + 2)
                        p_sb = ppool.tile([128, 512], F32R, name="ptile", tag="p")
                        nc.scalar.activation(
                            p_sb[:], s_tiles.pop(t)[:],
                            mybir.ActivationFunctionType.Exp, scale=ISQRT_E,
                        )
                        # rowsum accumulates on the vector engine instead of
                        # burning a PE matmul per t-block
                        p_in = p_sb[:].bitcast(F32)
                        if t == 0:
                            nc.vector.tensor_copy(p_acc[:], p_in)
                        else:
                            nc.vector.tensor_add(p_acc[:], p_acc[:], p_in)
                        for me in range(NKD):
                            msl = slice(me * 128, (me + 1) * 128)
                            nc.tensor.matmul(
                                o_ps[:, me, :], xn_sb[:, t, msl], p_sb[:],
                                start=(t == 0), stop=(t == NT - 1),
                                skip_group_check=True,
                            )

                    # scalar engine moves z out of PSUM (frees banks for the
                    # next window while the vector engine handles rowsums)
                    zt_sb = otpool.tile([128, NKD, 512], F32R, name="zt", tag="ot")
                    for me in range(NKD):
                        nc.scalar.copy(zt_sb[:, me, :], o_ps[:, me, :])

                    # prefetch the next (batch, window)'s q' -- across batch
                    # seams too -- so the PE stays busy while the
                    # normalization chain below runs on DVE/ACT
                    if w + 1 < NW:
                        qt_next = emit_qt(b, w + 1)
                    elif b + 1 < B:
                        qt_next = emit_qt(b + 1, 0)
                    else:
                        qt_next = None

                    # per-query rowsums straight into column layout:
                    # rtp[:, j] = p_acc[:, j-block].T @ ones -- tiny N=1 fp32
                    # matmuls (fp32r forbids N=1; 4 cyc/row x 1 row is free)
                    rtp = mm_ps.tile([128, 4], F32, name="rtp", tag="mm")
                    for j in range(4):
                        nc.tensor.matmul(
                            rtp[:, j:j + 1],
                            p_acc[:, j * 128:(j + 1) * 128], ones_f32[:],
                            start=True, stop=True,
                        )
                    rraw = rpool.tile([128, 4], F32, name="rraw", tag="rraw")
                    nc.vector.tensor_copy(rraw[:], rtp[:])
                    rcol = rpool.tile([128, 4], F32, name="rcol", tag="rc")
                    nc.vector.reciprocal(rcol[:], rraw[:])

                    # output projection for this window; its psum lives in the
                    # o_ps pool slot (freed above by the zt copies) so the
                    # mm pool's scores/qt slots never wait on the slower
                    # normalization drain below
                    proj_ps = o_ps_pool.tile([128, NKD, 512], F32,
                                             name="projps", tag="ops")
                    for j in range(4):
                        jsl = slice(j * 128, (j + 1) * 128)
                        for me in range(NKD):
                            nc.tensor.matmul(
                                proj_ps[:, j, :], zt_sb[:, me, jsl], g_sb[:, me, :],
                                start=(me == 0), stop=(me == NKD - 1),
                            )
                        po_sb = opool.tile([128, 512], F32, name="po", tag="po")
                        # normalization: per-partition scalar on the
                        # PSUM->SBUF move
                        nc.vector.tensor_scalar(
                            po_sb[:], proj_ps[:, j, :], rcol[:, j:j + 1], None,
                            mybir.AluOpType.mult,
                        )
                        row0 = b * S + w * 512 + j * 128
                        nc.sync.dma_start(
                            out=out_ext[row0:row0 + 128, :], in_=po_sb[:]
                        )
                    qt_sb = qt_next

    nc.compile()
    return nc


def _get_nc():
    if "nc" not in _CACHE:
        _CACHE["nc"] = _build()
    return _CACHE["nc"]


def _numpy_fallback(emb, Wq, bq, Wk, bk, Wv, bv, Wp, bp):
    x = emb.astype(np.float64)
    out = np.zeros((B, S, D), dtype=np.float64)
    for h in range(H):
        q = x @ Wq[h].astype(np.float64) + bq[h]
        k = x @ Wk[h].astype(np.float64) + bk[h]
        v = x @ Wv[h].astype(np.float64) + bv[h]
        for b in range(B):
            sc = (q[b] @ k[b].T) / np.sqrt(E)
            sc -= sc.max(axis=1, keepdims=True)
            p = np.exp(sc)
            p /= p.sum(axis=1, keepdims=True)
            out[b] += (p @ v[b]) @ Wp[h * E:(h + 1) * E].astype(np.float64)
    return (out + bp).astype(np.float32)


def _run(inputs, trace=False):
    emb = np.ascontiguousarray(inputs["emb_input"], dtype=np.float32)
    Wq = np.ascontiguousarray(inputs["Wq"], dtype=np.float32)
    Wk = np.ascontiguousarray(inputs["Wk"], dtype=np.float32)
    Wv = np.ascontiguousarray(inputs["Wv"], dtype=np.float32)
    Wp = np.ascontiguousarray(inputs["Wp"], dtype=np.float32)
    bq = np.asarray(inputs["bq"], dtype=np.float32)
    bk = np.asarray(inputs["bk"], dtype=np.float32)
    bv = np.asarray(inputs["bv"], dtype=np.float32)
    bp = np.asarray(inputs["bp"], dtype=np.float32)

    if np.any(bq) or np.any(bk) or np.any(bv):
        # the device program folds Wq/Wk and Wv/Wp together, which assumes
        # the q/k/v biases are structurally zero (problem spec fill=zeros);
        # anything else falls back to host math
        return _numpy_fallback(emb, Wq, bq, Wk, bk, Wv, bv, Wp, bp), None

    xt = np.ascontiguousarray(emb.transpose(2, 0, 1).reshape(D, B * S))
    xn = emb.reshape(B * S, D)
    in_maps = []
    for h in range(H):
        wq64 = Wq[h].astype(np.float64)
        wk64 = Wk[h].astype(np.float64)
        wv64 = Wv[h].astype(np.float64)
        wp64 = Wp[h * E:(h + 1) * E, :].astype(np.float64)
        in_maps.append({
            "xt": xt,
            "xn": xn,
            "m": (wq64 @ wk64.T).astype(np.float32),
            "g": (wv64 @ wp64).astype(np.float32),
        })

    nc = _get_nc()
    try:
        res = run_bass_kernel_spmd(nc, in_maps, list(range(H)), trace=trace)
    except Exception:
        res = run_bass_kernel_spmd(nc, in_maps, list(range(H)), trace=trace)
    acc = res.results[0]["out"].astype(np.float32, copy=True)
    for h in range(1, H):
        acc += res.results[h]["out"]
    out = acc.reshape(B, S, D) + bp[None, None, :]
    return out.astype(np.float32), res


def kernel(**inputs):
    out, _ = _run(inputs, trace=False)
    return out
